# revision 20
# baseline (speedup 1.0000x reference)
"""Trainium2 Bass kernel for the gated-attention module (8 NeuronCores, SPMD).

Module math (per reference):
    qsig = sigmoid(qs); ksig = sigmoid(ks_p)
    vsig = sigmoid(f)*tanh(c),  (c,f) = split(sigmoid(vs) @ vq_w.T + vq_b)
    q = qsig * LN(query @ ql_w.T + ql_b)        [S,B,H]
    k = ksig * key ; v = vsig * value
    out[q,b,:] = softmax(q_h . k_h / sqrt(H)) @ v_h   (per head h)

Kernel strategy:
  - Shard (batch, query-block): core = b*4 + qc handles query rows
    [qc*512:(qc+1)*512] of batch b, with full K/V for that batch.
  - Host-side constant folding of the tiny gate vectors (pure functions of
    the module *parameters*, no data dependence):
        G  = qsig*ksig*ln_g/sqrt(H); Bv = qsig*ksig*ln_b/sqrt(H); vsig
    so on-device  q_eff = norm(y)*G + Bv,  scores = q_eff . key  (no key
    gating needed),  out = vsig * (P @ value).
  - bf16 matmul operands, pre-transposed on host into the contraction
    layouts the PE needs (q^T, k^T, w^T); fp32 psum accumulation for the
    q_linear and PV reductions; fp32 LN statistics and output.
  - Scores are computed transposed (k on partitions) so softmax's P feeds
    the PV matmul directly with no P transpose; the softmax denominator
    comes from a ones-column appended to V. exp() needs no max-subtract:
    |scores| <~ 0.4 (LN output scaled by sigmoid-gates/32), far from
    overflow.
  - Heads are processed in pairs with score matmuls interleaved at lhsT
    base-partitions 0/64 so the PE row-groups run them concurrently
    (contract dim is only 64).
"""

import sys

sys.path.insert(0, "/opt/trn_rl_repo")

import numpy as np
import ml_dtypes

S = 2048
B = 2
H = 1024
H2 = 2 * H
NH = 16
HD = 64
TQ = S // 4  # 512 query rows per core
NKC = S // 128  # 16 k-chunks
SCALE = float(np.sqrt(H))
EPS = 1e-12

_CACHE = {}


def _build_bass():
    import concourse.bacc as bacc
    import concourse.bass as bass
    import concourse.tile as tile
    from concourse import mybir
    from concourse.masks import make_identity

    f32 = mybir.dt.float32
    bf16 = mybir.dt.bfloat16
    AF = mybir.ActivationFunctionType
    ALU = mybir.AluOpType

    nc = bacc.Bacc(None, target_bir_lowering=False)

    qt_d = nc.dram_tensor("qt", [H2, TQ], bf16, kind="ExternalInput")
    kt_d = nc.dram_tensor("kt", [H, S], bf16, kind="ExternalInput")
    wt_d = nc.dram_tensor("wt", [H2, H], bf16, kind="ExternalInput")
    v_d = nc.dram_tensor("vaug", [NKC, 128, NH, HD + 1], bf16, kind="ExternalInput")
    qlb_d = nc.dram_tensor("qlb", [H], f32, kind="ExternalInput")
    g_d = nc.dram_tensor("gvec", [H], f32, kind="ExternalInput")
    bv_d = nc.dram_tensor("bvec", [H], f32, kind="ExternalInput")
    out_d = nc.dram_tensor("out", [TQ, H], f32, kind="ExternalOutput")

    def bcast(dram_handle):
        # replicate a [H] dram vector across all 128 partitions
        ap = dram_handle[:]
        return bass.AP(tensor=ap.tensor, offset=ap.offset, ap=[[0, 128], [1, H]])

    with tile.TileContext(nc) as tc:
        with tc.tile_pool(name="persist", bufs=1) as persist:
            id_bf = persist.tile([128, 128], bf16)
            make_identity(nc, id_bf)
            id_f32 = persist.tile([128, 128], f32)
            make_identity(nc, id_f32)
            eps_t = persist.tile([128, 1], f32)
            nc.vector.memset(eps_t[:], EPS)

            warm_sb = persist.tile([128, 512], bf16)
            nc.vector.memset(warm_sb[:], 0.5)

            qlb_r = persist.tile([128, H], f32)
            g_r = persist.tile([128, H], f32)
            bv_r = persist.tile([128, H], f32)
            nc.gpsimd.dma_start(out=qlb_r[:], in_=bcast(qlb_d))
            nc.gpsimd.dma_start(out=g_r[:], in_=bcast(g_d))
            nc.gpsimd.dma_start(out=bv_r[:], in_=bcast(bv_d))

            # K^T tiles: kt_sb[p, dc, :] = key[:, dc*128+p] (host pre-transposed)
            kt_sb = persist.tile([128, 8, S], bf16)
            # V (+ ones column): vsb[p, kc, h, m] = vaug[kc, p, h, m]
            vsb = persist.tile([128, NKC, NH, HD + 1], bf16)

            # q_eff^T lives here: [o partitions, o-chunk, t]
            qeT = persist.tile([128, 8, TQ], bf16)
            # final output staging, one tile per 128-row query block
            outsb = [
                persist.tile([128, H], f32, name=f"outsb{i}", tag=f"outsb{i}")
                for i in range(4)
            ]

            # ---------------- phase 1+2: q_linear + LayerNorm ----------------
            with tc.tile_pool(name="ph2", bufs=1) as ph2:
                qt_sb = ph2.tile([128, 16, TQ], bf16)
                wt_sb = ph2.tile([128, 16, H], bf16)
                # qt/wt chunks alternate between the two HWDGE rings so group
                # g's operands arrive together and matmuls can chase the DMA
                # stream; phase-3 operands (kt, vsb) queue behind them
                def qt_chunk(g4):
                    return (
                        qt_sb[:, g4 * 4 : (g4 + 1) * 4, :],
                        qt_d[g4 * 512 : (g4 + 1) * 512, :].rearrange(
                            "(ic p) t -> p ic t", p=128
                        ),
                    )

                def wt_chunk(g4):
                    return (
                        wt_sb[:, g4 * 4 : (g4 + 1) * 4, :],
                        wt_d[g4 * 512 : (g4 + 1) * 512, :].rearrange(
                            "(ic p) o -> p ic o", p=128
                        ),
                    )

                for g4 in range(4):
                    eng_a = nc.sync if g4 % 2 == 0 else nc.scalar
                    eng_b = nc.scalar if g4 % 2 == 0 else nc.sync
                    o, i = qt_chunk(g4)
                    eng_a.dma_start(out=o, in_=i)
                    o, i = wt_chunk(g4)
                    eng_b.dma_start(out=o, in_=i)
                for half in range(2):
                    nc.sync.dma_start(
                        out=kt_sb[:, half * 4 : (half + 1) * 4, :],
                        in_=kt_d[half * 512 : (half + 1) * 512, :].rearrange(
                            "(dc p) k -> p dc k", p=128
                        ),
                    )
                nc.scalar.dma_start(
                    out=vsb[:], in_=v_d[:].rearrange("c p h m -> p c h m")
                )
                ysb = [
                    ph2.tile([128, H], f32, name=f"ysb{i}", tag=f"ysb{i}")
                    for i in range(4)
                ]
                mv = [
                    ph2.tile([128, 2], f32, name=f"mv{i}", tag=f"mv{i}")
                    for i in range(4)
                ]
                rst = [
                    ph2.tile([128, 1], f32, name=f"rst{i}", tag=f"rst{i}")
                    for i in range(4)
                ]

                # PE pre-warm: dummy matmuls keep the HAM activity monitor
                # busy while the first qt/wt chunks stream in, so the real
                # q_linear matmuls start at the full 2.4 GHz clock
                with tc.tile_pool(name="warm", bufs=1, space="PSUM") as warm:
                    wp = warm.tile([128, 512], f32)
                    for _ in range(22):
                        nc.tensor.matmul(
                            wp[:], lhsT=warm_sb[:, 0:128], rhs=warm_sb[:],
                            start=True, stop=True,
                        )

                with (
                    tc.tile_pool(name="ylin", bufs=2, space="PSUM") as ylin,
                    tc.tile_pool(name="tpq", bufs=2, space="PSUM") as tpq,
                    tc.tile_pool(name="warm2", bufs=1, space="PSUM") as warm2,
                    tc.tile_pool(name="st", bufs=4) as st_pool,
                    tc.tile_pool(name="qe", bufs=4) as qe_pool,
                ):
                    # all q_linear matmuls back-to-back on the PE; LN chains
                    # (DVE/ACT) trail behind each chunk's eviction
                    for tc4 in range(4):
                        y_ps = ylin.tile([128, 2, 512], f32)
                        for ic in range(16):
                            lhsT = qt_sb[:, ic, tc4 * 128 : (tc4 + 1) * 128]
                            for oc in range(2):
                                nc.tensor.matmul(
                                    y_ps[:, oc, :],
                                    lhsT=lhsT,
                                    rhs=wt_sb[:, ic, oc * 512 : (oc + 1) * 512],
                                    start=(ic == 0),
                                    stop=(ic == 15),
                                )
                        nc.vector.tensor_add(
                            ysb[tc4][:],
                            y_ps[:].rearrange("p a b -> p (a b)"),
                            qlb_r[:],
                        )
                        st = st_pool.tile([128, 2, 6], f32)
                        nc.vector.bn_stats(st[:, 0, :], ysb[tc4][:, 0:512])
                        nc.vector.bn_stats(st[:, 1, :], ysb[tc4][:, 512:1024])
                        nc.vector.bn_aggr(mv[tc4][:], st[:])
                    # keep the PE warm while the LN chain (DVE/ACT) catches up
                    wp2 = warm2.tile([128, 512], f32)
                    for _ in range(12):
                        nc.tensor.matmul(
                            wp2[:], lhsT=warm_sb[:, 0:128], rhs=warm_sb[:],
                            start=True, stop=True,
                        )
                    # batched rstd: all Ln, then all Exp (one ACT table set)
                    lv = [
                        st_pool.tile([128, 1], f32, name=f"lv{i}", tag=f"lv{i}")
                        for i in range(4)
                    ]
                    for tc4 in range(4):
                        nc.scalar.activation(
                            lv[tc4][:], mv[tc4][:, 1:2], AF.Ln, bias=eps_t[:]
                        )
                    for tc4 in range(4):
                        nc.scalar.activation(
                            rst[tc4][:], lv[tc4][:], AF.Exp, scale=-0.5
                        )
                    qe = []
                    for tc4 in range(4):
                        nc.vector.tensor_scalar(
                            out=ysb[tc4][:],
                            in0=ysb[tc4][:],
                            scalar1=mv[tc4][:, 0:1],
                            scalar2=rst[tc4][:],
                            op0=ALU.subtract,
                            op1=ALU.mult,
                        )
                        nc.vector.tensor_mul(ysb[tc4][:], ysb[tc4][:], g_r[:])
                        q = qe_pool.tile([128, H], bf16, name=f"qe{tc4}")
                        nc.vector.tensor_add(q[:], ysb[tc4][:], bv_r[:])
                        qe.append(q)
                    # o-chunk-major transposes: head pair 0's q_eff^T finishes
                    # first so attention can begin while later chunks transpose
                    for oc8 in range(8):
                        for tc4 in range(4):
                            tp = tpq.tile([128, 128], bf16)
                            nc.tensor.transpose(
                                tp[:],
                                qe[tc4][:, oc8 * 128 : (oc8 + 1) * 128],
                                id_bf[:],
                            )
                            nc.vector.tensor_copy(
                                qeT[:, oc8, tc4 * 128 : (tc4 + 1) * 128], tp[:]
                            )

            # ---------------- phase 3: attention, head pairs ----------------
            with (
                tc.tile_pool(name="sc", bufs=2, space="PSUM") as sc_pool,
                tc.tile_pool(name="pv", bufs=1, space="PSUM") as pv_pool,
                tc.tile_pool(name="tp2", bufs=2, space="PSUM") as tp2_pool,
                tc.tile_pool(name="pt", bufs=3) as pt_pool,
                tc.tile_pool(name="pvsb", bufs=2) as pvsb_pool,
                tc.tile_pool(name="rec", bufs=4) as rec_pool,
            ):
                for hp in range(8):
                    pv = pv_pool.tile([65, 2, 512], f32)
                    for kc in range(NKC):
                        ks = slice(kc * 128, (kc + 1) * 128)
                        sc = sc_pool.tile([128, 2, 512], f32)
                        # adjacent MMs at base-partition 0/64 row-pack
                        nc.tensor.matmul(
                            sc[:, 0, :],
                            lhsT=kt_sb[0:64, hp, ks],
                            rhs=qeT[0:64, hp, :],
                            start=True,
                            stop=True,
                        )
                        nc.tensor.matmul(
                            sc[:, 1, :],
                            lhsT=kt_sb[64:128, hp, ks],
                            rhs=qeT[64:128, hp, :],
                            start=True,
                            stop=True,
                        )
                        pt = pt_pool.tile([128, 2, 512], bf16)
                        nc.scalar.activation(
                            pt[:].rearrange("p a b -> p (a b)"),
                            sc[:].rearrange("p a b -> p (a b)"),
                            AF.Exp,
                        )
                        for e in range(2):
                            nc.tensor.matmul(
                                pv[:, e, :],
                                lhsT=vsb[:, kc, 2 * hp + e, :],
                                rhs=pt[:, e, :],
                                start=(kc == 0),
                                stop=(kc == NKC - 1),
                            )
                    pvsb = pvsb_pool.tile([65, 2, 512], f32)
                    nc.vector.tensor_copy(
                        pvsb[:].rearrange("p a b -> p (a b)"),
                        pv[:].rearrange("p a b -> p (a b)"),
                    )
                    for e in range(2):
                        h = 2 * hp + e
                        for qs in range(4):
                            tp2 = tp2_pool.tile([128, 65], f32)
                            nc.tensor.transpose(
                                tp2[:],
                                pvsb[:, e, qs * 128 : (qs + 1) * 128],
                                id_f32[0:65, 0:65],
                            )
                            rec = rec_pool.tile([128, 1], f32)
                            nc.vector.reciprocal(rec[:], tp2[:, 64:65])
                            nc.vector.tensor_scalar_mul(
                                outsb[qs][:, h * HD : (h + 1) * HD],
                                in0=tp2[:, 0:64],
                                scalar1=rec[:],
                            )
                for qs in range(4):
                    nc.sync.dma_start(
                        out=out_d[qs * 128 : (qs + 1) * 128, :], in_=outsb[qs][:]
                    )

    nc.compile()
    return nc


def _host_prep(query, key, value, qs, ks_p, vs, vq_w, vq_b, ql_w, ql_b, ln_g, ln_b):
    """Fold the gate-parameter math on host; build per-core device inputs."""
    bf16 = ml_dtypes.bfloat16

    def sig(x):
        return 1.0 / (1.0 + np.exp(-x.astype(np.float64)))

    qsig = sig(qs).reshape(H)
    ksig = sig(ks_p).reshape(H)
    hg = sig(vs).reshape(H) @ vq_w.astype(np.float64).T + vq_b.astype(np.float64)
    c, f = hg[:H], hg[H:]
    vsig = (1.0 / (1.0 + np.exp(-f))) * np.tanh(c)
    gg = qsig * ksig / SCALE
    G = (gg * ln_g.astype(np.float64)).astype(np.float32)
    Bv = (gg * ln_b.astype(np.float64)).astype(np.float32)
    vsig = vsig.astype(np.float32)
    qlb = ql_b.astype(np.float32)

    wt_bf = np.ascontiguousarray(ql_w.astype(bf16).T)  # [2H, H]

    per_batch = {}
    for b in range(B):
        kt_bf = np.ascontiguousarray(key[:, b, :].astype(bf16).T)  # [H, S]
        # fold the vsig output gate into V (out = vsig * (P@V) = P @ (vsig*V));
        # the appended ones column yields the softmax denominator
        v_b = (value[:, b, :] * vsig[None, :]).reshape(NKC, 128, NH, HD)
        vaug = np.ascontiguousarray(
            np.concatenate(
                [v_b, np.ones((NKC, 128, NH, 1), np.float32)], axis=-1
            ).astype(bf16)
        )
        per_batch[b] = (kt_bf, vaug)

    in_maps = []
    for core in range(8):
        b, qc = core // 4, core % 4
        qt_bf = np.ascontiguousarray(
            query[qc * TQ : (qc + 1) * TQ, b, :].astype(bf16).T
        )  # [2H, TQ]
        kt_bf, vaug = per_batch[b]
        in_maps.append(
            {
                "qt": qt_bf,
                "kt": kt_bf,
                "wt": wt_bf,
                "vaug": vaug,
                "qlb": qlb,
                "gvec": G,
                "bvec": Bv,
            }
        )
    return in_maps


def kernel(**inputs):
    from concourse.bass_utils import run_bass_kernel_spmd

    if "nc" not in _CACHE:
        _CACHE["nc"] = _build_bass()
    nc = _CACHE["nc"]

    in_maps = _host_prep(**inputs)
    res = run_bass_kernel_spmd(nc, in_maps, core_ids=list(range(8)))

    out = np.empty((S, B, H), np.float32)
    for core in range(8):
        b, qc = core // 4, core % 4
        out[qc * TQ : (qc + 1) * TQ, b, :] = res.results[core]["out"]
    return out


# revision 21
# speedup vs baseline: 1.0172x; 1.0172x over previous
"""Trainium2 Bass kernel for the gated-attention module (8 NeuronCores, SPMD).

Module math (per reference):
    qsig = sigmoid(qs); ksig = sigmoid(ks_p)
    vsig = sigmoid(f)*tanh(c),  (c,f) = split(sigmoid(vs) @ vq_w.T + vq_b)
    q = qsig * LN(query @ ql_w.T + ql_b)        [S,B,H]
    k = ksig * key ; v = vsig * value
    out[q,b,:] = softmax(q_h . k_h / sqrt(H)) @ v_h   (per head h)

Kernel strategy:
  - Shard (batch, query-block): core = b*4 + qc handles query rows
    [qc*512:(qc+1)*512] of batch b, with full K/V for that batch.
  - Host-side constant folding of the tiny gate vectors (pure functions of
    the module *parameters*, no data dependence):
        G  = qsig*ksig*ln_g/sqrt(H); Bv = qsig*ksig*ln_b/sqrt(H); vsig
    so on-device  q_eff = norm(y)*G + Bv,  scores = q_eff . key  (no key
    gating needed),  out = vsig * (P @ value).
  - bf16 matmul operands, pre-transposed on host into the contraction
    layouts the PE needs (q^T, k^T, w^T); fp32 psum accumulation for the
    q_linear and PV reductions; fp32 LN statistics and output.
  - Scores are computed transposed (k on partitions) so softmax's P feeds
    the PV matmul directly with no P transpose; the softmax denominator
    comes from a ones-column appended to V. exp() needs no max-subtract:
    |scores| <~ 0.4 (LN output scaled by sigmoid-gates/32), far from
    overflow.
  - Heads are processed in pairs with score matmuls interleaved at lhsT
    base-partitions 0/64 so the PE row-groups run them concurrently
    (contract dim is only 64).
"""

import sys

sys.path.insert(0, "/opt/trn_rl_repo")

import numpy as np
import ml_dtypes

S = 2048
B = 2
H = 1024
H2 = 2 * H
NH = 16
HD = 64
TQ = S // 4  # 512 query rows per core
NKC = S // 128  # 16 k-chunks
SCALE = float(np.sqrt(H))
EPS = 1e-12

_CACHE = {}


def _build_bass():
    import concourse.bacc as bacc
    import concourse.bass as bass
    import concourse.tile as tile
    from concourse import mybir
    from concourse.masks import make_identity

    f32 = mybir.dt.float32
    bf16 = mybir.dt.bfloat16
    AF = mybir.ActivationFunctionType
    ALU = mybir.AluOpType

    nc = bacc.Bacc(None, target_bir_lowering=False)

    qt_d = nc.dram_tensor("qt", [H2, TQ], bf16, kind="ExternalInput")
    kt_d = nc.dram_tensor("kt", [H, S], bf16, kind="ExternalInput")
    wt_d = nc.dram_tensor("wt", [H2, H], bf16, kind="ExternalInput")
    v_d = nc.dram_tensor("vaug", [NKC, 128, NH, HD + 1], bf16, kind="ExternalInput")
    qlb_d = nc.dram_tensor("qlb", [H], f32, kind="ExternalInput")
    g_d = nc.dram_tensor("gvec", [H], f32, kind="ExternalInput")
    bv_d = nc.dram_tensor("bvec", [H], f32, kind="ExternalInput")
    out_d = nc.dram_tensor("out", [TQ, H], f32, kind="ExternalOutput")

    def bcast(dram_handle):
        # replicate a [H] dram vector across all 128 partitions
        ap = dram_handle[:]
        return bass.AP(tensor=ap.tensor, offset=ap.offset, ap=[[0, 128], [1, H]])

    with tile.TileContext(nc) as tc:
        with tc.tile_pool(name="persist", bufs=1) as persist:
            id_bf = persist.tile([128, 128], bf16)
            make_identity(nc, id_bf)
            id_f32 = persist.tile([128, 128], f32)
            make_identity(nc, id_f32)
            eps_t = persist.tile([128, 1], f32)
            nc.vector.memset(eps_t[:], EPS)

            warm_sb = persist.tile([128, 512], bf16)
            nc.vector.memset(warm_sb[:], 0.5)

            qlb_r = persist.tile([128, H], f32)
            g_r = persist.tile([128, H], f32)
            bv_r = persist.tile([128, H], f32)
            nc.gpsimd.dma_start(out=qlb_r[:], in_=bcast(qlb_d))
            nc.gpsimd.dma_start(out=g_r[:], in_=bcast(g_d))
            nc.gpsimd.dma_start(out=bv_r[:], in_=bcast(bv_d))

            # K^T tiles: kt_sb[p, dc, :] = key[:, dc*128+p] (host pre-transposed)
            kt_sb = persist.tile([128, 8, S], bf16)
            # V (+ ones column): vsb[p, kc, h, m] = vaug[kc, p, h, m]
            vsb = persist.tile([128, NKC, NH, HD + 1], bf16)

            # q_eff^T lives here: [o partitions, o-chunk, t]
            qeT = persist.tile([128, 8, TQ], bf16)
            # final output staging, one tile per 128-row query block
            outsb = [
                persist.tile([128, H], f32, name=f"outsb{i}", tag=f"outsb{i}")
                for i in range(4)
            ]

            # ---------------- phase 1+2: q_linear + LayerNorm ----------------
            with tc.tile_pool(name="ph2", bufs=1) as ph2:
                qt_sb = ph2.tile([128, 16, TQ], bf16)
                wt_sb = ph2.tile([128, 16, H], bf16)
                # qt/wt chunks alternate between the two HWDGE rings so group
                # g's operands arrive together and matmuls can chase the DMA
                # stream; phase-3 operands (kt, vsb) queue behind them
                def qt_chunk(g4):
                    return (
                        qt_sb[:, g4 * 4 : (g4 + 1) * 4, :],
                        qt_d[g4 * 512 : (g4 + 1) * 512, :].rearrange(
                            "(ic p) t -> p ic t", p=128
                        ),
                    )

                def wt_chunk(g4):
                    return (
                        wt_sb[:, g4 * 4 : (g4 + 1) * 4, :],
                        wt_d[g4 * 512 : (g4 + 1) * 512, :].rearrange(
                            "(ic p) o -> p ic o", p=128
                        ),
                    )

                for g4 in range(4):
                    eng_a = nc.sync if g4 % 2 == 0 else nc.scalar
                    eng_b = nc.scalar if g4 % 2 == 0 else nc.sync
                    o, i = qt_chunk(g4)
                    eng_a.dma_start(out=o, in_=i)
                    o, i = wt_chunk(g4)
                    eng_b.dma_start(out=o, in_=i)
                for half in range(2):
                    nc.sync.dma_start(
                        out=kt_sb[:, half * 4 : (half + 1) * 4, :],
                        in_=kt_d[half * 512 : (half + 1) * 512, :].rearrange(
                            "(dc p) k -> p dc k", p=128
                        ),
                    )
                nc.scalar.dma_start(
                    out=vsb[:], in_=v_d[:].rearrange("c p h m -> p c h m")
                )
                ysb = [
                    ph2.tile([128, H], f32, name=f"ysb{i}", tag=f"ysb{i}")
                    for i in range(4)
                ]
                mv = [
                    ph2.tile([128, 2], f32, name=f"mv{i}", tag=f"mv{i}")
                    for i in range(4)
                ]
                rst = [
                    ph2.tile([128, 1], f32, name=f"rst{i}", tag=f"rst{i}")
                    for i in range(4)
                ]

                # PE pre-warm: dummy matmuls keep the HAM activity monitor
                # busy while the first qt/wt chunks stream in, so the real
                # q_linear matmuls start at the full 2.4 GHz clock
                with tc.tile_pool(name="warm", bufs=1, space="PSUM") as warm:
                    wp = warm.tile([128, 512], f32)
                    for _ in range(22):
                        nc.tensor.matmul(
                            wp[:], lhsT=warm_sb[:, 0:128], rhs=warm_sb[:],
                            start=True, stop=True,
                        )

                with (
                    tc.tile_pool(name="ylin", bufs=2, space="PSUM") as ylin,
                    tc.tile_pool(name="tpq", bufs=2, space="PSUM") as tpq,
                    tc.tile_pool(name="warm2", bufs=1, space="PSUM") as warm2,
                    tc.tile_pool(name="st", bufs=4) as st_pool,
                    tc.tile_pool(name="qe", bufs=4) as qe_pool,
                ):
                    # per-chunk: q_linear matmuls, bias+stats on DVE, then the
                    # full LN chain; chunk tc4's q_eff^T transposes are emitted
                    # after chunk tc4+1's matmuls so the LN latency hides under
                    # PE work and the PE never idles long enough to rethrottle
                    wp2 = warm2.tile([128, 512], f32)
                    qe = [None] * 4
                    lv = [
                        st_pool.tile([128, 1], f32, name=f"lv{i}", tag=f"lv{i}")
                        for i in range(4)
                    ]

                    def emit_mms(tc4):
                        y_ps = ylin.tile([128, 2, 512], f32)
                        for ic in range(16):
                            lhsT = qt_sb[:, ic, tc4 * 128 : (tc4 + 1) * 128]
                            for oc in range(2):
                                nc.tensor.matmul(
                                    y_ps[:, oc, :],
                                    lhsT=lhsT,
                                    rhs=wt_sb[:, ic, oc * 512 : (oc + 1) * 512],
                                    start=(ic == 0),
                                    stop=(ic == 15),
                                )
                            if tc4 == 0:
                                # dummy matmul fills the PE while the next
                                # qt/wt chunk is still streaming in
                                nc.tensor.matmul(
                                    wp2[:], lhsT=warm_sb[:, 0:128],
                                    rhs=warm_sb[:], start=True, stop=True,
                                )
                        return y_ps

                    def emit_ln(tc4, y_ps):
                        nc.vector.tensor_add(
                            ysb[tc4][:],
                            y_ps[:].rearrange("p a b -> p (a b)"),
                            qlb_r[:],
                        )
                        st = st_pool.tile([128, 2, 6], f32)
                        nc.vector.bn_stats(st[:, 0, :], ysb[tc4][:, 0:512])
                        nc.vector.bn_stats(st[:, 1, :], ysb[tc4][:, 512:1024])
                        nc.vector.bn_aggr(mv[tc4][:], st[:])
                        nc.scalar.activation(
                            lv[tc4][:], mv[tc4][:, 1:2], AF.Ln, bias=eps_t[:]
                        )
                        nc.scalar.activation(
                            rst[tc4][:], lv[tc4][:], AF.Exp, scale=-0.5
                        )
                        nc.vector.tensor_scalar(
                            out=ysb[tc4][:],
                            in0=ysb[tc4][:],
                            scalar1=mv[tc4][:, 0:1],
                            scalar2=rst[tc4][:],
                            op0=ALU.subtract,
                            op1=ALU.mult,
                        )
                        nc.vector.tensor_mul(ysb[tc4][:], ysb[tc4][:], g_r[:])
                        q = qe_pool.tile([128, H], bf16, name=f"qe{tc4}")
                        nc.vector.tensor_add(q[:], ysb[tc4][:], bv_r[:])
                        qe[tc4] = q

                    def emit_transposes(tc4, with_dummies=False):
                        for oc8 in range(8):
                            tp = tpq.tile([128, 128], bf16)
                            nc.tensor.transpose(
                                tp[:],
                                qe[tc4][:, oc8 * 128 : (oc8 + 1) * 128],
                                id_bf[:],
                            )
                            nc.vector.tensor_copy(
                                qeT[:, oc8, tc4 * 128 : (tc4 + 1) * 128], tp[:]
                            )
                            if with_dummies and oc8 % 2 == 1:
                                # transpose-mode doesn't register as PE
                                # activity for the clock gate; keep it warm
                                nc.tensor.matmul(
                                    wp2[:], lhsT=warm_sb[:, 0:128],
                                    rhs=warm_sb[:], start=True, stop=True,
                                )

                    y_prev = emit_mms(0)
                    emit_ln(0, y_prev)
                    y_prev = emit_mms(1)
                    emit_ln(1, y_prev)
                    emit_transposes(0)
                    y_prev = emit_mms(2)
                    emit_ln(2, y_prev)
                    emit_transposes(1)
                    y_prev = emit_mms(3)
                    emit_ln(3, y_prev)
                    emit_transposes(2, with_dummies=True)
                    emit_transposes(3, with_dummies=True)

            # ---------------- phase 3: attention, head pairs ----------------
            with (
                tc.tile_pool(name="sc", bufs=2, space="PSUM") as sc_pool,
                tc.tile_pool(name="pv", bufs=1, space="PSUM") as pv_pool,
                tc.tile_pool(name="tp2", bufs=2, space="PSUM") as tp2_pool,
                tc.tile_pool(name="pt", bufs=3) as pt_pool,
                tc.tile_pool(name="pvsb", bufs=2) as pvsb_pool,
                tc.tile_pool(name="rec", bufs=4) as rec_pool,
            ):
                for hp in range(8):
                    pv = pv_pool.tile([65, 2, 512], f32)
                    for kc in range(NKC):
                        ks = slice(kc * 128, (kc + 1) * 128)
                        sc = sc_pool.tile([128, 2, 512], f32)
                        # adjacent MMs at base-partition 0/64 row-pack
                        nc.tensor.matmul(
                            sc[:, 0, :],
                            lhsT=kt_sb[0:64, hp, ks],
                            rhs=qeT[0:64, hp, :],
                            start=True,
                            stop=True,
                        )
                        nc.tensor.matmul(
                            sc[:, 1, :],
                            lhsT=kt_sb[64:128, hp, ks],
                            rhs=qeT[64:128, hp, :],
                            start=True,
                            stop=True,
                        )
                        pt = pt_pool.tile([128, 2, 512], bf16)
                        nc.scalar.activation(
                            pt[:].rearrange("p a b -> p (a b)"),
                            sc[:].rearrange("p a b -> p (a b)"),
                            AF.Exp,
                        )
                        for e in range(2):
                            nc.tensor.matmul(
                                pv[:, e, :],
                                lhsT=vsb[:, kc, 2 * hp + e, :],
                                rhs=pt[:, e, :],
                                start=(kc == 0),
                                stop=(kc == NKC - 1),
                            )
                    pvsb = pvsb_pool.tile([65, 2, 512], f32)
                    nc.vector.tensor_copy(
                        pvsb[:].rearrange("p a b -> p (a b)"),
                        pv[:].rearrange("p a b -> p (a b)"),
                    )
                    for e in range(2):
                        h = 2 * hp + e
                        for qs in range(4):
                            tp2 = tp2_pool.tile([128, 65], f32)
                            nc.tensor.transpose(
                                tp2[:],
                                pvsb[:, e, qs * 128 : (qs + 1) * 128],
                                id_f32[0:65, 0:65],
                            )
                            rec = rec_pool.tile([128, 1], f32)
                            nc.vector.reciprocal(rec[:], tp2[:, 64:65])
                            nc.vector.tensor_scalar_mul(
                                outsb[qs][:, h * HD : (h + 1) * HD],
                                in0=tp2[:, 0:64],
                                scalar1=rec[:],
                            )
                for qs in range(4):
                    nc.sync.dma_start(
                        out=out_d[qs * 128 : (qs + 1) * 128, :], in_=outsb[qs][:]
                    )

    nc.compile()
    return nc


def _host_prep(query, key, value, qs, ks_p, vs, vq_w, vq_b, ql_w, ql_b, ln_g, ln_b):
    """Fold the gate-parameter math on host; build per-core device inputs."""
    bf16 = ml_dtypes.bfloat16

    def sig(x):
        return 1.0 / (1.0 + np.exp(-x.astype(np.float64)))

    qsig = sig(qs).reshape(H)
    ksig = sig(ks_p).reshape(H)
    hg = sig(vs).reshape(H) @ vq_w.astype(np.float64).T + vq_b.astype(np.float64)
    c, f = hg[:H], hg[H:]
    vsig = (1.0 / (1.0 + np.exp(-f))) * np.tanh(c)
    gg = qsig * ksig / SCALE
    G = (gg * ln_g.astype(np.float64)).astype(np.float32)
    Bv = (gg * ln_b.astype(np.float64)).astype(np.float32)
    vsig = vsig.astype(np.float32)
    qlb = ql_b.astype(np.float32)

    wt_bf = np.ascontiguousarray(ql_w.astype(bf16).T)  # [2H, H]

    per_batch = {}
    for b in range(B):
        kt_bf = np.ascontiguousarray(key[:, b, :].astype(bf16).T)  # [H, S]
        # fold the vsig output gate into V (out = vsig * (P@V) = P @ (vsig*V));
        # the appended ones column yields the softmax denominator
        v_b = (value[:, b, :] * vsig[None, :]).reshape(NKC, 128, NH, HD)
        vaug = np.ascontiguousarray(
            np.concatenate(
                [v_b, np.ones((NKC, 128, NH, 1), np.float32)], axis=-1
            ).astype(bf16)
        )
        per_batch[b] = (kt_bf, vaug)

    in_maps = []
    for core in range(8):
        b, qc = core // 4, core % 4
        qt_bf = np.ascontiguousarray(
            query[qc * TQ : (qc + 1) * TQ, b, :].astype(bf16).T
        )  # [2H, TQ]
        kt_bf, vaug = per_batch[b]
        in_maps.append(
            {
                "qt": qt_bf,
                "kt": kt_bf,
                "wt": wt_bf,
                "vaug": vaug,
                "qlb": qlb,
                "gvec": G,
                "bvec": Bv,
            }
        )
    return in_maps


def kernel(**inputs):
    from concourse.bass_utils import run_bass_kernel_spmd

    if "nc" not in _CACHE:
        _CACHE["nc"] = _build_bass()
    nc = _CACHE["nc"]

    in_maps = _host_prep(**inputs)
    res = run_bass_kernel_spmd(nc, in_maps, core_ids=list(range(8)))

    out = np.empty((S, B, H), np.float32)
    for core in range(8):
        b, qc = core // 4, core % 4
        out[qc * TQ : (qc + 1) * TQ, b, :] = res.results[core]["out"]
    return out
